# revision 27
# baseline (speedup 1.0000x reference)
"""Llama GQA attention (B=1, S=2048, D=4096, H=32, KV=8, HD=128) on 8 Trainium2
NeuronCores, tensor-parallel over heads.

Sharding: core c owns Q heads 4c..4c+3 and KV head c (GQA groups align with the
8 KV heads). Wq/Wk/Wv are column-sliced, Wo row-sliced; each core produces a
full-shape partial output (bf16) and the host sums the 8 partials (row-parallel
TP all-reduce done at unshard time).

Device kernel layout strategy: the host passes X^T so every projection matmul
produces transposed activations [head_dim=128 partitions, seq free]:
    Q^T/K^T/V^T = W.T @ X^T   (lhsT = W slice, rhs = X^T tile)
Scores are computed transposed, S^T[k, q] = K^T_tile.T @ Q^T, the exp runs on
the scalar engine (PSUM->SBUF), and the PV matmul consumes E^T directly
(lhsT = V natural tile); o_proj consumes O^T directly as lhsT. RoPE
rotate-half is done with two small SBUF->SBUF DMAs (DVE lanes are
partition-locked; DMA is not) plus one DVE multiply against a sign-folded
sin table (no PE matmul). 1/sqrt(HD) is folded into Wq on the host.
Causality: k-tiles above the diagonal are skipped, and on the four
diagonal k-tiles the q range is trimmed to q >= j*128 (N = 512,384,256,128),
with one shared [128,128] triangular multiplicative mask on each tile's
leading 128 q columns (exp never overflows: scores are O(10) here, so
max-subtraction is unnecessary). Diagonal scores share PSUM banks so a
single exp covers each two-tile group.

Softmax denominator: instead of a PE ones-matmul per k-tile (which costs as
much as PV again), the DVE accumulates esum[q] = sum over processed e-tiles
in fp32 SBUF (2 adds per pair), the scalar engine casts it to bf16 right
when it completes, and ONE bf16 ones@esum matmul per (q-chunk, head)
broadcasts the denominator across partitions. This cuts attention PE time
by ~30%. Each head's den/recip/mul tail is deferred one slot so the den
matmul never stalls the in-order PE stream on the DVE chain.

Slot schedule: qc0 and qc1 heads interleave at the phase boundary (qc1's
bigger units fill qc0's Act-bound pipeline-fill gaps), later chunks run
head-sequential with o_proj blocks of the oldest finished chunk as PE-dense
filler between heads (attention alone is Act-exp-bound once den is off the
PE), and the last two chunks' o_proj trails as a pure-PE tail.

Scheduling (what got this from 526us to ~426us and then ~409us):
 - All DRAM tensors are host-packed so every DMA has 4-8KB contiguous
   per-partition runs (descriptor size = per-partition run length).
 - 18 warm-up matmuls on a memset tile run from t~6us so the PE HAM clock
   gate (4/8 -> 8/8 at ~3.4us of sustained activity) is already open when
   the first real matmul's operands land.
 - boot DMA is split: boot0 (consts + a=0 weights, ~330KB) unblocks the
   first matmul; boot1a/boot1b ride behind, interleaved with the X quarters.
   The wq/wkv streams are chopped into ~0.5MB pieces spread across the
   first x-tile iterations so the startup dependency set isn't diluted by
   bulk prefetch (xin pool depth 6 bounds in-flight X).
 - Attention processes full k-tile PAIRS: scores land in a [128,1024] PSUM
   tile and one scalar-engine exp covers both tiles, amortizing the ACTIVATE
   fixed cost (352cyc).
 - Attention is qc-outer/head-inner and o_proj for the finished 512-row block
   is emitted right after, so phases 2+3 form one dense PE stream; PSUM is
   exactly 8 banks: 2x score-pair + 2x (ops|den).
 - epool/rbpool/espool live on the RIGHT side of SBUF so the first exp's
   e-tile never WARs against phase-1 regions still being read by the last
   RoPE epilogue (this WAR caused a 2.5us PE bubble + 10us HAM re-throttle
   at the phase boundary).
 - Each RoPE epilogue runs its PSUM readers (casts, V transposes) FIRST and
   defers the SBUF-only muls/adds, so acc banks free early for attention.
 - softmax reciprocal uses the single-op DVE reciprocal_approx_fast.
 - o_proj results stage through [128,2048] tiles (512KB DMAs, 4KB runs) and
   partials are written as bf16 (host sums in f64); the very last staging
   tile is split into two 256KB DMAs so the NEFF drain starts earlier.

Matmul operands are bf16 (PE runs 4x faster than true fp32; accumulation stays
fp32 in PSUM); softmax statistics and RoPE trig stay fp32. fp8 was evaluated
and rejected: e4m3 quantization noise on any of the big projections costs
3-6e-2 rel err vs the 2e-2 budget.
"""

import numpy as np

S = 2048
D = 4096
HD = 128
HQ = 4            # Q heads per core
P = 128
QC = 512          # q-chunk (matmul moving free dim)
SCALING = float(HD) ** -0.5
N_CORES = 8
WARM_MM = 18      # PE warm-up matmuls (HAM un-throttle before real work)

MM_MODE = "bf16"

_PROG_CACHE = {}


def _mm_np_dtype(mm_mode="bf16"):
    import ml_dtypes
    return ml_dtypes.bfloat16


# boot column layout (bf16): wq a=0 | wkv a=0 | wq a=1..3 | wkv a=1..3 |
#   id | ones | tri | wkv a=4..15   (weights first: the consts aren't
#   needed until the first epilogue, so only 192KB gates the first matmul)
BOOT0_END = HQ * HD + 2 * HD                  # 768
BOOT1A_END = BOOT0_END + 3 * HQ * HD + 3 * 2 * HD   # 3072
CONST0 = BOOT1A_END                           # id|ones|tri
BOOT_COLS = CONST0 + 3 * P + 12 * 2 * HD      # 6528


def _build_program(mm_mode: str = "bf16", s: int = S):
    import concourse.tile as tile
    from concourse import bacc, mybir

    F32 = mybir.dt.float32
    F32R = mybir.dt.float32r
    BF16 = mybir.dt.bfloat16
    MMDT = BF16
    H2 = HD // 2

    nqc = s // QC           # q chunks
    nkt = s // P            # k tiles
    kd = D // P             # contraction tiles over model dim

    kd4 = kd // 4           # packed X groups of 4 contraction tiles

    nc = bacc.Bacc("TRN2", target_bir_lowering=False, debug=False)
    xt = nc.dram_tensor("xt", [nqc * kd4, P, 4 * QC], MMDT,
                        kind="ExternalInput")
    wq = nc.dram_tensor("wq", [P, kd * HQ * HD], MMDT, kind="ExternalInput")
    wkv = nc.dram_tensor("wkv", [P, kd * 2 * HD], MMDT, kind="ExternalInput")
    wo = nc.dram_tensor("wo", [P, HQ * D], MMDT, kind="ExternalInput")
    cost = nc.dram_tensor("cost", [HD, s], MMDT, kind="ExternalInput")
    sint = nc.dram_tensor("sint", [HD, s], MMDT, kind="ExternalInput")
    boot = nc.dram_tensor("boot", [P, BOOT_COLS], MMDT, kind="ExternalInput")
    out = nc.dram_tensor("out", [s, D], MMDT, kind="ExternalOutput")

    wq_r = wq.ap().rearrange("p (a m) -> p a m", m=HQ * HD)  # [128, kd, 512]
    wkv_r = wkv.ap().rearrange("p (a m) -> p a m", m=2 * HD)
    wo_r = wo.ap().rearrange("p (h d) -> p h d", d=D)        # [128, HQ, D]
    out_r = out.ap().rearrange("(a p) d -> a p d", p=P)      # [s/128, 128, D]

    def boot_wq_off(a):
        return 0 if a == 0 else BOOT0_END + (a - 1) * HQ * HD

    def boot_wkv_off(a):
        if a == 0:
            return HQ * HD
        if a < 4:
            return BOOT0_END + 3 * HQ * HD + (a - 1) * 2 * HD
        return CONST0 + 3 * P + (a - 4) * 2 * HD

    with tile.TileContext(nc) as tc:
        with tc.tile_pool(name="persist", bufs=1) as persist:
            qT = [persist.tile([HD, s], MMDT, name=f"qT{h}") for h in range(HQ)]
            kT = persist.tile([HD, s], MMDT, name="kT")
            v_sb = persist.tile([P, nkt, HD], MMDT, name="v_sb")
            oT = [persist.tile([HD, s], MMDT, name=f"oT{h}") for h in range(HQ)]
            wo_sb = persist.tile([P, HQ, D], MMDT, name="wo_sb")
            ones32 = persist.tile([P, P], F32, name="ones32")
            boot_sb = persist.tile([P, BOOT_COLS], MMDT, name="boot_sb")
            id_sb = boot_sb[:, CONST0:CONST0 + P]
            ones_sb = boot_sb[:, CONST0 + P:CONST0 + 2 * P]
            tri_sb = boot_sb[:, CONST0 + 2 * P:CONST0 + 3 * P]

            # ---------------- Phase 1: QKV projection + RoPE ----------------
            with (
                tc.tile_pool(name="ph1", bufs=1) as ph1,
                tc.tile_pool(name="xin", bufs=6) as xin,
                tc.tile_pool(name="ropes", bufs=3) as ropes,
                tc.tile_pool(name="accp", bufs=1, space="PSUM") as accp,
                tc.tile_pool(name="rqp", bufs=2, space="PSUM") as rqp,
            ):
                cos_sb = ph1.tile([HD, s], MMDT, name="cos_sb")
                sin_sb = ph1.tile([HD, s], MMDT, name="sin_sb")
                vT_sb = ph1.tile([HD, s], MMDT, name="vT_sb")
                wq_sb = ph1.tile([P, kd, HQ * HD], MMDT, name="wq_sb")
                wkv_sb = ph1.tile([P, kd, 2 * HD], MMDT, name="wkv_sb")

                # PE warm-up: HAM opens the clock gate after ~3.4us of
                # sustained activity; run it on a memset tile while the
                # boot DMA is in flight so real matmuls start at 2.4GHz.
                warm_sb = ph1.tile([P, QC], MMDT, name="warm_sb")
                nc.vector.memset(warm_sb, 0.0)
                nc.vector.memset(ones32, 1.0)
                for _ in range(WARM_MM):
                    wp = rqp.tile([P, QC], F32, name="wp", tag="rq")
                    nc.tensor.matmul(wp, lhsT=warm_sb[:, :P], rhs=warm_sb,
                                     start=True, stop=True)

                for ci, qc in enumerate(range(nqc)):
                    sl = slice(qc * QC, (qc + 1) * QC)
                    accs = [
                        accp.tile([P, QC], F32, name=f"acc{t}", tag=f"acc{t}")
                        for t in range(6)
                    ]
                    for a4 in range(kd4):
                        xt_t = xin.tile([P, 4 * QC], MMDT, name="xt_t")
                        if ci == 0 and a4 == 0:
                            # startup: interleave the first-matmul dependency
                            # set (boot0 + first x quarter) ahead of the rest
                            nc.sync.dma_start(boot_sb[:, :BOOT0_END],
                                              boot.ap()[:, :BOOT0_END])
                            nc.sync.dma_start(xt_t[:, :QC],
                                              xt.ap()[0][:, :QC])
                            nc.sync.dma_start(
                                boot_sb[:, BOOT0_END:BOOT1A_END],
                                boot.ap()[:, BOOT0_END:BOOT1A_END])
                            nc.sync.dma_start(xt_t[:, QC:2 * QC],
                                              xt.ap()[0][:, QC:2 * QC])
                            nc.sync.dma_start(boot_sb[:, BOOT1A_END:],
                                              boot.ap()[:, BOOT1A_END:])
                            nc.sync.dma_start(xt_t[:, 2 * QC:],
                                              xt.ap()[0][:, 2 * QC:])
                        else:
                            nc.sync.dma_start(xt_t, xt.ap()[qc * kd4 + a4])
                        if ci == 0 and a4 <= 6:
                            # rest of wq in 0.5MB pieces between the x tiles
                            c = 4 + 4 * a4
                            nc.sync.dma_start(wq_sb[:, c:c + 4, :],
                                              wq_r[:, c:c + 4, :])
                        if ci == 0 and a4 in (2, 3):
                            c = 16 + 8 * (a4 - 2)
                            nc.sync.dma_start(wkv_sb[:, c:c + 8, :],
                                              wkv_r[:, c:c + 8, :])
                        if ci == 0 and a4 == 5:
                            nc.sync.dma_start(cos_sb, cost.ap())
                            nc.sync.dma_start(sin_sb, sint.ap())
                        if ci == 1 and a4 in (1, 3, 5, 7):
                            # phase-2/3 constants, spread out mid-stream
                            h = (a4 - 1) // 2
                            nc.sync.dma_start(wo_sb[:, h, :], wo_r[:, h, :])
                        for j in range(4):
                            a = 4 * a4 + j
                            rhs = xt_t[:, j * QC:(j + 1) * QC]
                            if a < 4:
                                qo = boot_wq_off(a)
                                qa = boot_sb[:, qo:qo + HQ * HD]
                            else:
                                qa = wq_sb[:, a, :]
                            wsl = [qa[:, h * HD:(h + 1) * HD]
                                   for h in range(HQ)]
                            if a < 16:
                                kb = boot_wkv_off(a)
                                wsl += [boot_sb[:, kb:kb + HD],
                                        boot_sb[:, kb + HD:kb + 2 * HD]]
                            else:
                                wsl += [wkv_sb[:, a, 0:HD],
                                        wkv_sb[:, a, HD:]]
                            for t in range(6):
                                nc.tensor.matmul(
                                    accs[t], lhsT=wsl[t], rhs=rhs,
                                    start=(a == 0), stop=(a == kd - 1),
                                )
                    # RoPE epilogue: PSUM readers (casts, V transposes) run
                    # FIRST so the acc banks (aliased by the attention PSUM
                    # pool) free early; the SBUF-only muls/adds are deferred
                    # and overlap with whatever follows.
                    nc.scalar.copy(out=vT_sb[:, sl], in_=accs[5])
                    raws = []
                    for t in range(5):
                        raw = ropes.tile([P, QC], MMDT, name="raw",
                                         tag="raw", bufs=5)
                        if t % 2 == 1 or t == 4:
                            nc.scalar.copy(out=raw, in_=accs[t])
                        else:
                            nc.vector.tensor_copy(out=raw, in_=accs[t])
                        raws.append(raw)
                    # V^T -> V natural layout for this chunk's 4 seq tiles
                    for st in range(4 * qc, 4 * qc + 4):
                        tp = rqp.tile([P, P], MMDT, name="tp", tag="rq")
                        nc.tensor.transpose(tp, vT_sb[:, st * P:(st + 1) * P],
                                            id_sb)
                        nc.vector.tensor_copy(out=v_sb[:, st, :], in_=tp)
                    if ci == nqc - 1:
                        # pad the PE through the last epilogue's lull so the
                        # HAM clock gate stays at 8/8 into the attention
                        # phase (it re-throttles after ~3.4us of idle)
                        for _ in range(14):
                            wpad = rqp.tile([P, QC], F32, name="wpad",
                                            tag="rq")
                            nc.tensor.matmul(wpad, lhsT=ones_sb,
                                             rhs=boot_sb[:, :QC],
                                             start=True, stop=True)
                    # SBUF-only tail: rotate-half via two small SBUF->SBUF
                    # DMAs (DVE lanes are partition-locked, DMA is not),
                    # then one mul against the sign-folded sin (rows 0:64
                    # negated host-side) -- no PE matmul needed.
                    for t in range(5):
                        rot = ropes.tile([P, QC], MMDT, name="rot",
                                         tag="rot", bufs=5)
                        nc.sync.dma_start(rot[0:H2], raws[t][H2:P])
                        nc.sync.dma_start(rot[H2:P], raws[t][0:H2])
                        tmp = ropes.tile([P, QC], F32, name="tmp",
                                         tag="tmp", bufs=5)
                        nc.vector.tensor_mul(out=tmp, in0=rot,
                                             in1=sin_sb[:, sl])
                        dst = qT[t] if t < HQ else kT
                        nc.vector.tensor_mul(out=dst[:, sl], in0=raws[t],
                                             in1=cos_sb[:, sl])
                        nc.vector.tensor_add(out=dst[:, sl], in0=dst[:, sl],
                                             in1=tmp)

            # ---- Phases 2+3 software-pipelined: attention + o_proj ----
            # PSUM: sp tag = 2x [128,1024] (score units AND o_proj dd-pairs,
            #       4 banks), od tag = 2x [128,1024] opd ops|den (4 banks).
            # epool/rbpool/espool are on the RIGHT side of SBUF: no WAR
            # against phase-1 regions still read by the last RoPE tail.
            # Each head's den/recip/mul tail is DEFERRED until after the
            # next head's units (the den matmul otherwise stalls the
            # in-order PE stream on the DVE esum chain), and o_proj blocks
            # of chunk qc-1 are interleaved between chunk qc's heads so the
            # PE has dense filler while the scalar engine works through the
            # exps (attention alone is Act-bound once den is off the PE).
            with (
                tc.tile_pool(name="ppsum", bufs=2, space="PSUM") as ppsum,
                tc.tile_pool(name="epool", bufs=3, side="right") as epool,
                tc.tile_pool(name="rbpool", bufs=2, side="right") as rbpool,
                tc.tile_pool(name="espool", bufs=2, side="right") as espool,
                tc.tile_pool(name="res", bufs=4) as res,
            ):
                def emit_oproj(st, last):
                    # o_proj for one finished 128-row seq block (both halves)
                    for half in range(2):
                        r = res.tile([P, 4 * QC], MMDT, name="r")
                        for k in range(2):
                            ddp = 2 * half + k
                            op = ppsum.tile([P, 2 * QC], F32, name="op",
                                            tag="sp")
                            for i in range(2):
                                c0 = ddp * 2 * QC + i * QC
                                for h in range(HQ):
                                    nc.tensor.matmul(
                                        op[:, i * QC:(i + 1) * QC],
                                        lhsT=oT[h][:, st * P:(st + 1) * P],
                                        rhs=wo_sb[:, h, c0:c0 + QC],
                                        start=(h == 0), stop=(h == HQ - 1),
                                    )
                            dst = r[:, k * 2 * QC:(k + 1) * 2 * QC]
                            if last and half == 1:
                                # final tiles: split copies across both
                                # engines and DMA each 256KB piece as soon
                                # as it's staged (shortens the drain)
                                nc.vector.tensor_copy(out=dst[:, :QC],
                                                      in_=op[:, :QC])
                                nc.scalar.copy(out=dst[:, QC:],
                                               in_=op[:, QC:])
                                nc.sync.dma_start(
                                    out_r[st, :,
                                          (half * 2 + k) * 2 * QC:
                                          (half * 2 + k + 1) * 2 * QC],
                                    dst)
                            elif k == 0:
                                nc.vector.tensor_copy(out=dst, in_=op)
                            else:
                                # k1 on the scalar engine: balancing the
                                # PSUM-read copies across engines keeps the
                                # DVE FIFO short for the recip/mul tails
                                nc.scalar.copy(out=dst, in_=op)
                        if not (last and half == 1):
                            nc.sync.dma_start(
                                out_r[st, :,
                                      half * 4 * QC:(half + 1) * 4 * QC],
                                r)

                def emit_units(qc, h, prev_tail=None):
                    """Score units + exps + masks + esum + PVs for one
                    (q-chunk, head); returns the deferred tail closure.
                    prev_tail (the previous slot's den/recip/mul) is fired
                    just before the diagonal PVs: its den matmul fills the
                    PE's wait on the diagonal exp, and the recip/mul launch
                    earlier, releasing the opd ring sooner."""
                    sl = slice(qc * QC, (qc + 1) * QC)
                    nfull = 2 * qc          # full (unmasked) k-tile pairs
                    nunit = nfull + 2       # + 2 trimmed diagonal groups

                    def qk_pair(g):
                        sp = ppsum.tile([P, 2 * QC], F32, name="sp",
                                        tag="sp")
                        for i in range(2):
                            kt = 2 * g + i
                            nc.tensor.matmul(
                                sp[:, i * QC:(i + 1) * QC],
                                lhsT=kT[:, kt * P:(kt + 1) * P],
                                rhs=qT[h][:, sl], start=True, stop=True,
                            )
                        return sp

                    def qk_diag(which):
                        # diagonal k-tiles with q trimmed to q >= j*128:
                        # which=0: j=0 (N=512 at cols 0:512),
                        #          j=1 (N=384 at cols 512:896)
                        # which=1: j=2 (N=256 at cols 0:256),
                        #          j=3 (N=128 at cols 256:384, same bank --
                        #          two start=True writes to one bank only
                        #          clear has_written, data is preserved, so
                        #          one contiguous exp covers both)
                        sp = ppsum.tile([P, 2 * QC], F32, name="sp",
                                        tag="sp")
                        for j in (0, 1) if which == 0 else (2, 3):
                            n = QC - j * P
                            kt = 4 * qc + j
                            off = {0: 0, 1: QC, 2: 0, 3: 2 * P}[j]
                            nc.tensor.matmul(
                                sp[:, off:off + n],
                                lhsT=kT[:, kt * P:(kt + 1) * P],
                                rhs=qT[h][:, qc * QC + j * P:(qc + 1) * QC],
                                start=True, stop=True,
                                skip_group_check=(j == 3),
                            )
                        return sp

                    def unit_scores(u):
                        if u < nfull:
                            return qk_pair(u)
                        return qk_diag(u - nfull)

                    opd = ppsum.tile([P, 2 * QC], F32, name="opd",
                                     tag="od")
                    esum = espool.tile([P, QC], F32, name="esum")
                    sps = [unit_scores(0), unit_scores(1)]
                    for u in range(nunit):
                        if u + 2 < nunit:
                            sps.append(unit_scores(u + 2))
                        if u == nfull and prev_tail is not None:
                            prev_tail()
                            prev_tail = None
                        sp = sps[u]
                        e = epool.tile([P, 2 * QC], MMDT, name="e")
                        if u < nfull:
                            nc.scalar.activation(
                                out=e, in_=sp,
                                func=mybir.ActivationFunctionType.Exp,
                            )
                            if u == 0:
                                nc.vector.tensor_add(out=esum,
                                                     in0=e[:, :QC],
                                                     in1=e[:, QC:])
                            else:
                                nc.vector.tensor_add(out=esum, in0=esum,
                                                     in1=e[:, :QC])
                                nc.vector.tensor_add(out=esum, in0=esum,
                                                     in1=e[:, QC:])
                            for i in range(2):
                                kt = 2 * u + i
                                nc.tensor.matmul(
                                    opd[:, :QC], lhsT=v_sb[:, kt, :],
                                    rhs=e[:, i * QC:(i + 1) * QC],
                                    start=(u == 0 and i == 0), stop=False,
                                )
                        elif u == nfull:
                            # diag group 1: j=0 (N=512), j=1 (N=384);
                            # one exp covers both (cols 0:896 contiguous)
                            nc.scalar.activation(
                                out=e[:, :QC + 3 * P],
                                in_=sp[:, :QC + 3 * P],
                                func=mybir.ActivationFunctionType.Exp)
                            nc.vector.tensor_mul(out=e[:, :P],
                                                 in0=e[:, :P], in1=tri_sb)
                            nc.vector.tensor_mul(out=e[:, QC:QC + P],
                                                 in0=e[:, QC:QC + P],
                                                 in1=tri_sb)
                            if u == 0:
                                nc.vector.tensor_copy(out=esum,
                                                      in_=e[:, :QC])
                            else:
                                nc.vector.tensor_add(out=esum, in0=esum,
                                                     in1=e[:, :QC])
                            nc.vector.tensor_add(
                                out=esum[:, P:], in0=esum[:, P:],
                                in1=e[:, QC:QC + 3 * P])
                            nc.tensor.matmul(
                                opd[:, :QC], lhsT=v_sb[:, 4 * qc, :],
                                rhs=e[:, :QC],
                                start=(nfull == 0), stop=False)
                            nc.tensor.matmul(
                                opd[:, P:QC], lhsT=v_sb[:, 4 * qc + 1, :],
                                rhs=e[:, QC:QC + 3 * P],
                                start=False, stop=False)
                        else:
                            # diag group 2: j=2 (N=256 at 0:256), j=3
                            # (N=128 at 256:384); one exp covers both
                            nc.scalar.activation(
                                out=e[:, :3 * P], in_=sp[:, :3 * P],
                                func=mybir.ActivationFunctionType.Exp)
                            nc.vector.tensor_mul(out=e[:, :P],
                                                 in0=e[:, :P], in1=tri_sb)
                            nc.vector.tensor_mul(out=e[:, 2 * P:3 * P],
                                                 in0=e[:, 2 * P:3 * P],
                                                 in1=tri_sb)
                            nc.vector.tensor_add(
                                out=esum[:, 2 * P:], in0=esum[:, 2 * P:],
                                in1=e[:, :2 * P])
                            nc.vector.tensor_add(
                                out=esum[:, 3 * P:], in0=esum[:, 3 * P:],
                                in1=e[:, 2 * P:3 * P])
                            nc.tensor.matmul(
                                opd[:, 2 * P:QC],
                                lhsT=v_sb[:, 4 * qc + 2, :],
                                rhs=e[:, :2 * P], start=False, stop=False)
                            nc.tensor.matmul(
                                opd[:, 3 * P:QC],
                                lhsT=v_sb[:, 4 * qc + 3, :],
                                rhs=e[:, 2 * P:3 * P],
                                start=False, stop=True)

                    # cast the f32 esum to bf16 NOW (data just ready, Act
                    # queue short) so the den matmul in the deferred tail
                    # never waits on it; one rounding (~1e-3) on the den
                    esb = espool.tile([P, QC], MMDT, name="esb", tag="esb")
                    nc.scalar.copy(out=esb, in_=esum)

                    def tail():
                        # softmax denominator: broadcast colsum via one
                        # bf16 ones-matmul
                        nc.tensor.matmul(opd[:, QC:], lhsT=ones_sb,
                                         rhs=esb, start=True, stop=True)
                        rb = rbpool.tile([P, QC], F32, name="rb")
                        nc.vector.reciprocal_approx_fast(
                            out=rb, in_=opd[:, QC:])
                        nc.vector.tensor_mul(out=oT[h][:, sl],
                                             in0=opd[:, :QC], in1=rb)
                    return tail

                # Slot schedule: qc0 and qc1 heads interleave at the phase
                # boundary (qc1's bigger units fill qc0's Act-bound gaps --
                # there is no o_proj filler available yet); later chunks
                # run head-sequential with o_proj blocks of the oldest
                # fully-tailed chunk as PE-dense filler between heads.
                slots = []
                for h in range(HQ):
                    slots += [(0, h), (1, h)]
                for qc in range(2, nqc):
                    slots += [(qc, h) for h in range(HQ)]
                ost_fill = {8 + i: i for i in range(8)}  # slot -> o_proj st
                prev_tail = None
                for i, (qc, h) in enumerate(slots):
                    t = emit_units(qc, h, prev_tail)
                    if i in ost_fill:
                        emit_oproj(ost_fill[i], last=False)
                    prev_tail = t
                prev_tail()

                # trailing o_proj for the last two chunks' seq blocks
                for st in range(8, 16):
                    emit_oproj(st, last=(st == 15))

    nc.finalize()
    return nc


def _get_program(mm_mode: str = MM_MODE, s: int = S):
    key = (mm_mode, s)
    if key not in _PROG_CACHE:
        _PROG_CACHE[key] = _build_program(mm_mode, s)
    return _PROG_CACHE[key]


def make_in_maps(hidden_states, cos, sin, Wq, Wk, Wv, Wo, mm_mode=None):
    """Host-side sharding: slice per-core weights, transpose activations."""
    mdt = _mm_np_dtype()
    hidden_states = np.asarray(hidden_states, dtype=np.float32)
    cos = np.asarray(cos, dtype=np.float32)
    sin = np.asarray(sin, dtype=np.float32)
    Wq = np.asarray(Wq, dtype=np.float32)
    Wk = np.asarray(Wk, dtype=np.float32)
    Wv = np.asarray(Wv, dtype=np.float32)
    Wo = np.asarray(Wo, dtype=np.float32)

    s = hidden_states.shape[1]
    nqc, kd, kd4 = s // QC, D // P, D // P // 4
    XT = np.ascontiguousarray(hidden_states[0].T).astype(mdt)  # [D, s]
    # pack X^T so each (q-chunk, 4-contraction-tile) DMA has 4KB contiguous
    # per-partition runs: XP[qc*kd4+a4, p, j*QC+m] = XT[(4*a4+j)*P+p, qc*QC+m]
    XP = np.ascontiguousarray(
        XT.reshape(kd4, 4, P, nqc, QC).transpose(3, 0, 2, 1, 4)
        .reshape(nqc * kd4, P, 4 * QC))
    cT = np.ascontiguousarray(cos[0].T).astype(mdt)            # [HD, s]
    sT = np.ascontiguousarray(sin[0].T).astype(np.float32)
    # sign-folded sin for the DVE rotate-half: rows 0:63 negated
    sTf = np.concatenate([-sT[:HD // 2], sT[HD // 2:]], axis=0).astype(mdt)

    def pack_w(w):
        # [D, m] -> [P, kd*m]: partition p holds rows {a*P+p} concatenated
        m = w.shape[1]
        return np.ascontiguousarray(
            w.reshape(kd, P, m).transpose(1, 0, 2).reshape(P, kd * m))

    kk = np.arange(P)[:, None]
    qq = np.arange(P)[None, :]
    tri = (kk <= qq).astype(np.float32)
    consts = np.concatenate(
        [np.eye(P, dtype=np.float32), np.ones((P, P), np.float32), tri],
        axis=1).astype(mdt)

    in_maps = []
    for c in range(N_CORES):
        cw = c * HQ * HD
        # wo packed like the others but with P-row groups per head:
        # [P, HQ*D]: partition p holds head-h rows {h*P+p}
        wo_c = Wo[cw:cw + HQ * HD, :]
        wo_p = np.ascontiguousarray(
            wo_c.reshape(HQ, P, D).transpose(1, 0, 2).reshape(P, HQ * D))
        wq_p = pack_w(Wq[:, cw:cw + HQ * HD] * np.float32(SCALING)
                      ).astype(mdt)
        wkv_p = pack_w(np.concatenate(
            [Wk[:, c * HD:(c + 1) * HD], Wv[:, c * HD:(c + 1) * HD]],
            axis=1)).astype(mdt)
        m = HQ * HD
        boot = np.ascontiguousarray(np.concatenate(
            [wq_p[:, :m], wkv_p[:, :2 * HD],              # boot0 weights
             wq_p[:, m:4 * m], wkv_p[:, 2 * HD:8 * HD],   # boot1a
             consts, wkv_p[:, 8 * HD:32 * HD]],           # boot1b
            axis=1))
        assert boot.shape[1] == BOOT_COLS, boot.shape
        in_maps.append({
            "xt": XP,
            "wq": wq_p,
            "wkv": wkv_p,
            "wo": wo_p.astype(mdt),
            "cost": cT,
            "sint": sTf,
            "boot": boot,
        })
    return in_maps


def run_spmd(in_maps, s: int = S, trace: bool = False, **kw):
    from concourse.bass_utils import run_bass_kernel_spmd

    nc = _get_program(MM_MODE, s)
    return run_bass_kernel_spmd(
        nc, in_maps, core_ids=list(range(N_CORES)), trace=trace, **kw
    )


def kernel(hidden_states, cos, sin, Wq, Wk, Wv, Wo):
    in_maps = make_in_maps(hidden_states, cos, sin, Wq, Wk, Wv, Wo)
    s = np.asarray(hidden_states).shape[1]
    res = run_spmd(in_maps, s=s, trace=False)
    total = np.zeros((s, D), np.float64)
    for r in res.results:
        total += np.asarray(r["out"], dtype=np.float32)
    return total.astype(np.float32).reshape(1, s, D)


# revision 29
# speedup vs baseline: 1.0008x; 1.0008x over previous
"""Llama GQA attention (B=1, S=2048, D=4096, H=32, KV=8, HD=128) on 8 Trainium2
NeuronCores, tensor-parallel over heads.

Sharding: core c owns Q heads 4c..4c+3 and KV head c (GQA groups align with the
8 KV heads). Wq/Wk/Wv are column-sliced, Wo row-sliced; each core produces a
full-shape partial output (bf16) and the host sums the 8 partials (row-parallel
TP all-reduce done at unshard time).

Device kernel layout strategy: the host passes X^T so every projection matmul
produces transposed activations [head_dim=128 partitions, seq free]:
    Q^T/K^T/V^T = W.T @ X^T   (lhsT = W slice, rhs = X^T tile)
Scores are computed transposed, S^T[k, q] = K^T_tile.T @ Q^T, the exp runs on
the scalar engine (PSUM->SBUF), and the PV matmul consumes E^T directly
(lhsT = V natural tile); o_proj consumes O^T directly as lhsT. RoPE
rotate-half is done with two small SBUF->SBUF DMAs (DVE lanes are
partition-locked; DMA is not) plus one DVE multiply against a sign-folded
sin table (no PE matmul). 1/sqrt(HD) is folded into Wq on the host.
Causality: k-tiles above the diagonal are skipped, and on the four
diagonal k-tiles the q range is trimmed to q >= j*128 (N = 512,384,256,128),
with one shared [128,128] triangular multiplicative mask on each tile's
leading 128 q columns (exp never overflows: scores are O(10) here, so
max-subtraction is unnecessary). Diagonal scores share PSUM banks so a
single exp covers each two-tile group.

Softmax denominator: instead of a PE ones-matmul per k-tile (which costs as
much as PV again), the DVE accumulates esum[q] = sum over processed e-tiles
in fp32 SBUF (2 adds per pair), the scalar engine casts it to bf16 right
when it completes, and ONE bf16 ones@esum matmul per (q-chunk, head)
broadcasts the denominator across partitions. This cuts attention PE time
by ~30%. Each head's den/recip/mul tail is deferred one slot so the den
matmul never stalls the in-order PE stream on the DVE chain.

Slot schedule: qc0 and qc1 heads interleave at the phase boundary (qc1's
bigger units fill qc0's Act-bound pipeline-fill gaps), later chunks run
head-sequential with o_proj blocks of the oldest finished chunk as PE-dense
filler between heads (attention alone is Act-exp-bound once den is off the
PE), and the last two chunks' o_proj trails as a pure-PE tail.

Scheduling (what got this from 526us to ~426us and then ~409us):
 - All DRAM tensors are host-packed so every DMA has 4-8KB contiguous
   per-partition runs (descriptor size = per-partition run length).
 - 18 warm-up matmuls on a memset tile run from t~6us so the PE HAM clock
   gate (4/8 -> 8/8 at ~3.4us of sustained activity) is already open when
   the first real matmul's operands land.
 - boot DMA is split: boot0 (consts + a=0 weights, ~330KB) unblocks the
   first matmul; boot1a/boot1b ride behind, interleaved with the X quarters.
   The wq/wkv streams are chopped into ~0.5MB pieces spread across the
   first x-tile iterations so the startup dependency set isn't diluted by
   bulk prefetch (xin pool depth 6 bounds in-flight X).
 - Attention processes full k-tile PAIRS: scores land in a [128,1024] PSUM
   tile and one scalar-engine exp covers both tiles, amortizing the ACTIVATE
   fixed cost (352cyc).
 - Attention is qc-outer/head-inner and o_proj for the finished 512-row block
   is emitted right after, so phases 2+3 form one dense PE stream; PSUM is
   exactly 8 banks: 2x score-pair + 2x (ops|den).
 - epool/rbpool/espool live on the RIGHT side of SBUF so the first exp's
   e-tile never WARs against phase-1 regions still being read by the last
   RoPE epilogue (this WAR caused a 2.5us PE bubble + 10us HAM re-throttle
   at the phase boundary).
 - Each RoPE epilogue runs its PSUM readers (casts, V transposes) FIRST and
   defers the SBUF-only muls/adds, so acc banks free early for attention.
 - softmax reciprocal uses the single-op DVE reciprocal_approx_fast.
 - o_proj results stage through [128,2048] tiles (512KB DMAs, 4KB runs) and
   partials are written as bf16 (host sums in f64); the very last staging
   tile is split into two 256KB DMAs so the NEFF drain starts earlier.

Matmul operands are bf16 (PE runs 4x faster than true fp32; accumulation stays
fp32 in PSUM); softmax statistics and RoPE trig stay fp32. fp8 was evaluated
and rejected: e4m3 quantization noise on any of the big projections costs
3-6e-2 rel err vs the 2e-2 budget.
"""

import numpy as np

S = 2048
D = 4096
HD = 128
HQ = 4            # Q heads per core
P = 128
QC = 512          # q-chunk (matmul moving free dim)
SCALING = float(HD) ** -0.5
N_CORES = 8
WARM_MM = 18      # PE warm-up matmuls (HAM un-throttle before real work)

MM_MODE = "bf16"

_PROG_CACHE = {}


def _mm_np_dtype(mm_mode="bf16"):
    import ml_dtypes
    return ml_dtypes.bfloat16


# boot column layout (bf16): wq a=0 | wkv a=0 | wq a=1..3 | wkv a=1..3 |
#   id | ones | tri | wkv a=4..15   (weights first: the consts aren't
#   needed until the first epilogue, so only 192KB gates the first matmul)
BOOT0_END = HQ * HD + 2 * HD                  # 768
BOOT1A_END = BOOT0_END + 3 * HQ * HD + 3 * 2 * HD   # 3072
CONST0 = BOOT1A_END                           # id|ones|tri
BOOT_COLS = CONST0 + 3 * P + 12 * 2 * HD      # 6528


def _build_program(mm_mode: str = "bf16", s: int = S):
    import concourse.tile as tile
    from concourse import bacc, mybir

    F32 = mybir.dt.float32
    F32R = mybir.dt.float32r
    BF16 = mybir.dt.bfloat16
    MMDT = BF16
    H2 = HD // 2

    nqc = s // QC           # q chunks
    nkt = s // P            # k tiles
    kd = D // P             # contraction tiles over model dim

    kd4 = kd // 4           # packed X groups of 4 contraction tiles

    nc = bacc.Bacc("TRN2", target_bir_lowering=False, debug=False)
    xt = nc.dram_tensor("xt", [nqc * kd4, P, 4 * QC], MMDT,
                        kind="ExternalInput")
    wq = nc.dram_tensor("wq", [P, kd * HQ * HD], MMDT, kind="ExternalInput")
    wkv = nc.dram_tensor("wkv", [P, kd * 2 * HD], MMDT, kind="ExternalInput")
    wo = nc.dram_tensor("wo", [P, HQ * D], MMDT, kind="ExternalInput")
    cost = nc.dram_tensor("cost", [HD, s], MMDT, kind="ExternalInput")
    sint = nc.dram_tensor("sint", [HD, s], MMDT, kind="ExternalInput")
    boot = nc.dram_tensor("boot", [P, BOOT_COLS], MMDT, kind="ExternalInput")
    out = nc.dram_tensor("out", [s, D], MMDT, kind="ExternalOutput")

    wq_r = wq.ap().rearrange("p (a m) -> p a m", m=HQ * HD)  # [128, kd, 512]
    wkv_r = wkv.ap().rearrange("p (a m) -> p a m", m=2 * HD)
    wo_r = wo.ap().rearrange("p (h d) -> p h d", d=D)        # [128, HQ, D]
    out_r = out.ap().rearrange("(a p) d -> a p d", p=P)      # [s/128, 128, D]

    def boot_wq_off(a):
        return 0 if a == 0 else BOOT0_END + (a - 1) * HQ * HD

    def boot_wkv_off(a):
        if a == 0:
            return HQ * HD
        if a < 4:
            return BOOT0_END + 3 * HQ * HD + (a - 1) * 2 * HD
        return CONST0 + 3 * P + (a - 4) * 2 * HD

    with tile.TileContext(nc) as tc:
        with tc.tile_pool(name="persist", bufs=1) as persist:
            qT = [persist.tile([HD, s], MMDT, name=f"qT{h}") for h in range(HQ)]
            kT = persist.tile([HD, s], MMDT, name="kT")
            v_sb = persist.tile([P, nkt, HD], MMDT, name="v_sb")
            oT = [persist.tile([HD, s], MMDT, name=f"oT{h}") for h in range(HQ)]
            wo_sb = persist.tile([P, HQ, D], MMDT, name="wo_sb")
            ones32 = persist.tile([P, P], F32, name="ones32")
            boot_sb = persist.tile([P, BOOT_COLS], MMDT, name="boot_sb")
            id_sb = boot_sb[:, CONST0:CONST0 + P]
            ones_sb = boot_sb[:, CONST0 + P:CONST0 + 2 * P]
            tri_sb = boot_sb[:, CONST0 + 2 * P:CONST0 + 3 * P]

            # ---------------- Phase 1: QKV projection + RoPE ----------------
            with (
                tc.tile_pool(name="ph1", bufs=1) as ph1,
                tc.tile_pool(name="xin", bufs=6) as xin,
                tc.tile_pool(name="ropes", bufs=3) as ropes,
                tc.tile_pool(name="accp", bufs=1, space="PSUM") as accp,
                tc.tile_pool(name="rqp", bufs=2, space="PSUM") as rqp,
            ):
                cos_sb = ph1.tile([HD, s], MMDT, name="cos_sb")
                sin_sb = ph1.tile([HD, s], MMDT, name="sin_sb")
                vT_sb = ph1.tile([HD, s], MMDT, name="vT_sb")
                wq_sb = ph1.tile([P, kd, HQ * HD], MMDT, name="wq_sb")
                wkv_sb = ph1.tile([P, kd, 2 * HD], MMDT, name="wkv_sb")

                # PE warm-up: HAM opens the clock gate after ~3.4us of
                # sustained activity; run it on a memset tile while the
                # boot DMA is in flight so real matmuls start at 2.4GHz.
                warm_sb = ph1.tile([P, QC], MMDT, name="warm_sb")
                nc.vector.memset(warm_sb, 0.0)
                nc.vector.memset(ones32, 1.0)
                for _ in range(WARM_MM):
                    wp = rqp.tile([P, QC], F32, name="wp", tag="rq")
                    nc.tensor.matmul(wp, lhsT=warm_sb[:, :P], rhs=warm_sb,
                                     start=True, stop=True)

                for ci, qc in enumerate(range(nqc)):
                    sl = slice(qc * QC, (qc + 1) * QC)
                    accs = [
                        accp.tile([P, QC], F32, name=f"acc{t}", tag=f"acc{t}")
                        for t in range(6)
                    ]
                    for a4 in range(kd4):
                        xt_t = xin.tile([P, 4 * QC], MMDT, name="xt_t")
                        if ci == 0 and a4 == 0:
                            # startup: interleave the first-matmul dependency
                            # set (boot0 + first x quarter) ahead of the rest
                            nc.sync.dma_start(boot_sb[:, :BOOT0_END],
                                              boot.ap()[:, :BOOT0_END])
                            nc.sync.dma_start(xt_t[:, :QC],
                                              xt.ap()[0][:, :QC])
                            nc.sync.dma_start(
                                boot_sb[:, BOOT0_END:BOOT1A_END],
                                boot.ap()[:, BOOT0_END:BOOT1A_END])
                            nc.sync.dma_start(xt_t[:, QC:2 * QC],
                                              xt.ap()[0][:, QC:2 * QC])
                            nc.sync.dma_start(boot_sb[:, BOOT1A_END:],
                                              boot.ap()[:, BOOT1A_END:])
                            nc.sync.dma_start(xt_t[:, 2 * QC:],
                                              xt.ap()[0][:, 2 * QC:])
                        else:
                            nc.sync.dma_start(xt_t, xt.ap()[qc * kd4 + a4])
                        if ci == 0 and a4 <= 6:
                            # rest of wq in 0.5MB pieces between the x tiles
                            c = 4 + 4 * a4
                            nc.sync.dma_start(wq_sb[:, c:c + 4, :],
                                              wq_r[:, c:c + 4, :])
                        if ci == 0 and a4 in (2, 3):
                            c = 16 + 8 * (a4 - 2)
                            nc.sync.dma_start(wkv_sb[:, c:c + 8, :],
                                              wkv_r[:, c:c + 8, :])
                        if ci == 0 and a4 == 5:
                            nc.sync.dma_start(cos_sb, cost.ap())
                            nc.sync.dma_start(sin_sb, sint.ap())
                        if ci == 1 and a4 in (1, 3, 5, 7):
                            # phase-2/3 constants, spread out mid-stream
                            h = (a4 - 1) // 2
                            nc.sync.dma_start(wo_sb[:, h, :], wo_r[:, h, :])
                        for j in range(4):
                            a = 4 * a4 + j
                            rhs = xt_t[:, j * QC:(j + 1) * QC]
                            if a < 4:
                                qo = boot_wq_off(a)
                                qa = boot_sb[:, qo:qo + HQ * HD]
                            else:
                                qa = wq_sb[:, a, :]
                            wsl = [qa[:, h * HD:(h + 1) * HD]
                                   for h in range(HQ)]
                            if a < 16:
                                kb = boot_wkv_off(a)
                                wsl += [boot_sb[:, kb:kb + HD],
                                        boot_sb[:, kb + HD:kb + 2 * HD]]
                            else:
                                wsl += [wkv_sb[:, a, 0:HD],
                                        wkv_sb[:, a, HD:]]
                            for t in range(6):
                                nc.tensor.matmul(
                                    accs[t], lhsT=wsl[t], rhs=rhs,
                                    start=(a == 0), stop=(a == kd - 1),
                                )
                    # RoPE epilogue: PSUM readers (casts, V transposes) run
                    # FIRST so the acc banks (aliased by the attention PSUM
                    # pool) free early; the SBUF-only muls/adds are deferred
                    # and overlap with whatever follows.
                    nc.scalar.copy(out=vT_sb[:, sl], in_=accs[5])
                    raws = []
                    for t in range(5):
                        raw = ropes.tile([P, QC], MMDT, name="raw",
                                         tag="raw", bufs=5)
                        if t % 2 == 1 or t == 4:
                            nc.scalar.copy(out=raw, in_=accs[t])
                        else:
                            nc.vector.tensor_copy(out=raw, in_=accs[t])
                        raws.append(raw)
                    # V^T -> V natural layout for this chunk's 4 seq tiles
                    for st in range(4 * qc, 4 * qc + 4):
                        tp = rqp.tile([P, P], MMDT, name="tp", tag="rq")
                        nc.tensor.transpose(tp, vT_sb[:, st * P:(st + 1) * P],
                                            id_sb)
                        nc.vector.tensor_copy(out=v_sb[:, st, :], in_=tp)
                    if ci == nqc - 1:
                        # pad the PE through the last epilogue's lull so the
                        # HAM clock gate stays at 8/8 into the attention
                        # phase (it re-throttles after ~3.4us of idle)
                        for _ in range(14):
                            wpad = rqp.tile([P, QC], F32, name="wpad",
                                            tag="rq")
                            nc.tensor.matmul(wpad, lhsT=ones_sb,
                                             rhs=boot_sb[:, :QC],
                                             start=True, stop=True)
                    # SBUF-only tail: rotate-half via two small SBUF->SBUF
                    # DMAs (DVE lanes are partition-locked, DMA is not),
                    # then one mul against the sign-folded sin (rows 0:64
                    # negated host-side) -- no PE matmul needed.
                    for t in range(5):
                        rot = ropes.tile([P, QC], MMDT, name="rot",
                                         tag="rot", bufs=5)
                        nc.sync.dma_start(rot[0:H2], raws[t][H2:P])
                        nc.sync.dma_start(rot[H2:P], raws[t][0:H2])
                        tmp = ropes.tile([P, QC], F32, name="tmp",
                                         tag="tmp", bufs=5)
                        nc.vector.tensor_mul(out=tmp, in0=rot,
                                             in1=sin_sb[:, sl])
                        dst = qT[t] if t < HQ else kT
                        nc.vector.tensor_mul(out=dst[:, sl], in0=raws[t],
                                             in1=cos_sb[:, sl])
                        nc.vector.tensor_add(out=dst[:, sl], in0=dst[:, sl],
                                             in1=tmp)

            # ---- Phases 2+3 software-pipelined: attention + o_proj ----
            # PSUM: sp tag = 2x [128,1024] (score units AND o_proj dd-pairs,
            #       4 banks), od tag = 2x [128,1024] opd ops|den (4 banks).
            # epool/rbpool/espool are on the RIGHT side of SBUF: no WAR
            # against phase-1 regions still read by the last RoPE tail.
            # Each head's den/recip/mul tail is DEFERRED until after the
            # next head's units (the den matmul otherwise stalls the
            # in-order PE stream on the DVE esum chain), and o_proj blocks
            # of chunk qc-1 are interleaved between chunk qc's heads so the
            # PE has dense filler while the scalar engine works through the
            # exps (attention alone is Act-bound once den is off the PE).
            with (
                tc.tile_pool(name="ppsum", bufs=2, space="PSUM") as ppsum,
                tc.tile_pool(name="epool", bufs=3, side="right") as epool,
                tc.tile_pool(name="rbpool", bufs=2, side="right") as rbpool,
                tc.tile_pool(name="espool", bufs=2, side="right") as espool,
                tc.tile_pool(name="res", bufs=4) as res,
            ):
                def emit_oproj(st, last):
                    # o_proj for one finished 128-row seq block (both halves)
                    for half in range(2):
                        r = res.tile([P, 4 * QC], MMDT, name="r")
                        for k in range(2):
                            ddp = 2 * half + k
                            op = ppsum.tile([P, 2 * QC], F32, name="op",
                                            tag="sp")
                            for i in range(2):
                                c0 = ddp * 2 * QC + i * QC
                                for h in range(HQ):
                                    nc.tensor.matmul(
                                        op[:, i * QC:(i + 1) * QC],
                                        lhsT=oT[h][:, st * P:(st + 1) * P],
                                        rhs=wo_sb[:, h, c0:c0 + QC],
                                        start=(h == 0), stop=(h == HQ - 1),
                                    )
                            dst = r[:, k * 2 * QC:(k + 1) * 2 * QC]
                            if last and half == 1:
                                # final tiles: split copies across both
                                # engines and DMA each 256KB piece as soon
                                # as it's staged (shortens the drain)
                                nc.vector.tensor_copy(out=dst[:, :QC],
                                                      in_=op[:, :QC])
                                nc.scalar.copy(out=dst[:, QC:],
                                               in_=op[:, QC:])
                                nc.sync.dma_start(
                                    out_r[st, :,
                                          (half * 2 + k) * 2 * QC:
                                          (half * 2 + k + 1) * 2 * QC],
                                    dst)
                            elif k == 0:
                                nc.vector.tensor_copy(out=dst, in_=op)
                            else:
                                # k1 on the scalar engine: balancing the
                                # PSUM-read copies across engines keeps the
                                # DVE FIFO short for the recip/mul tails
                                nc.scalar.copy(out=dst, in_=op)
                        if not (last and half == 1):
                            nc.sync.dma_start(
                                out_r[st, :,
                                      half * 4 * QC:(half + 1) * 4 * QC],
                                r)

                def emit_units(qc, h, prev_tail=None):
                    """Score units + exps + masks + esum + PVs for one
                    (q-chunk, head); returns the deferred tail closure.
                    prev_tail (the previous slot's den/recip/mul) is fired
                    just before the diagonal PVs: its den matmul fills the
                    PE's wait on the diagonal exp, and the recip/mul launch
                    earlier, releasing the opd ring sooner."""
                    sl = slice(qc * QC, (qc + 1) * QC)
                    nfull = 2 * qc          # full (unmasked) k-tile pairs
                    nunit = nfull + 2       # + 2 trimmed diagonal groups

                    def qk_pair(g):
                        sp = ppsum.tile([P, 2 * QC], F32, name="sp",
                                        tag="sp")
                        for i in range(2):
                            kt = 2 * g + i
                            nc.tensor.matmul(
                                sp[:, i * QC:(i + 1) * QC],
                                lhsT=kT[:, kt * P:(kt + 1) * P],
                                rhs=qT[h][:, sl], start=True, stop=True,
                            )
                        return sp

                    def qk_diag(which):
                        # diagonal k-tiles with q trimmed to q >= j*128:
                        # which=0: j=0 (N=512 at cols 0:512),
                        #          j=1 (N=384 at cols 512:896)
                        # which=1: j=2 (N=256 at cols 0:256),
                        #          j=3 (N=128 at cols 256:384, same bank --
                        #          two start=True writes to one bank only
                        #          clear has_written, data is preserved, so
                        #          one contiguous exp covers both)
                        sp = ppsum.tile([P, 2 * QC], F32, name="sp",
                                        tag="sp")
                        for j in (0, 1) if which == 0 else (2, 3):
                            n = QC - j * P
                            kt = 4 * qc + j
                            off = {0: 0, 1: QC, 2: 0, 3: 2 * P}[j]
                            nc.tensor.matmul(
                                sp[:, off:off + n],
                                lhsT=kT[:, kt * P:(kt + 1) * P],
                                rhs=qT[h][:, qc * QC + j * P:(qc + 1) * QC],
                                start=True, stop=True,
                                skip_group_check=(j == 3),
                            )
                        return sp

                    def unit_scores(u):
                        if u < nfull:
                            return qk_pair(u)
                        return qk_diag(u - nfull)

                    opd = ppsum.tile([P, 2 * QC], F32, name="opd",
                                     tag="od")
                    esum = espool.tile([P, QC], F32, name="esum")
                    sps = [unit_scores(0), unit_scores(1)]
                    for u in range(nunit):
                        if u + 2 < nunit:
                            sps.append(unit_scores(u + 2))
                        if u == nfull and prev_tail is not None:
                            prev_tail()
                            prev_tail = None
                        sp = sps[u]
                        e = epool.tile([P, 2 * QC], MMDT, name="e")
                        if u < nfull:
                            nc.scalar.activation(
                                out=e, in_=sp,
                                func=mybir.ActivationFunctionType.Exp,
                            )
                            if u == 0:
                                nc.vector.tensor_add(out=esum,
                                                     in0=e[:, :QC],
                                                     in1=e[:, QC:])
                            else:
                                nc.vector.tensor_add(out=esum, in0=esum,
                                                     in1=e[:, :QC])
                                nc.vector.tensor_add(out=esum, in0=esum,
                                                     in1=e[:, QC:])
                            for i in range(2):
                                kt = 2 * u + i
                                nc.tensor.matmul(
                                    opd[:, :QC], lhsT=v_sb[:, kt, :],
                                    rhs=e[:, i * QC:(i + 1) * QC],
                                    start=(u == 0 and i == 0), stop=False,
                                )
                        elif u == nfull:
                            # diag group 1: j=0 (N=512), j=1 (N=384);
                            # one exp covers both (cols 0:896 contiguous)
                            nc.scalar.activation(
                                out=e[:, :QC + 3 * P],
                                in_=sp[:, :QC + 3 * P],
                                func=mybir.ActivationFunctionType.Exp)
                            nc.vector.tensor_mul(out=e[:, :P],
                                                 in0=e[:, :P], in1=tri_sb)
                            nc.vector.tensor_mul(out=e[:, QC:QC + P],
                                                 in0=e[:, QC:QC + P],
                                                 in1=tri_sb)
                            if u == 0:
                                nc.vector.tensor_copy(out=esum,
                                                      in_=e[:, :QC])
                            else:
                                nc.vector.tensor_add(out=esum, in0=esum,
                                                     in1=e[:, :QC])
                            nc.vector.tensor_add(
                                out=esum[:, P:], in0=esum[:, P:],
                                in1=e[:, QC:QC + 3 * P])
                            nc.tensor.matmul(
                                opd[:, :QC], lhsT=v_sb[:, 4 * qc, :],
                                rhs=e[:, :QC],
                                start=(nfull == 0), stop=False)
                            nc.tensor.matmul(
                                opd[:, P:QC], lhsT=v_sb[:, 4 * qc + 1, :],
                                rhs=e[:, QC:QC + 3 * P],
                                start=False, stop=False)
                        else:
                            # diag group 2: j=2 (N=256 at 0:256), j=3
                            # (N=128 at 256:384); one exp covers both
                            nc.scalar.activation(
                                out=e[:, :3 * P], in_=sp[:, :3 * P],
                                func=mybir.ActivationFunctionType.Exp)
                            nc.vector.tensor_mul(out=e[:, :P],
                                                 in0=e[:, :P], in1=tri_sb)
                            nc.vector.tensor_mul(out=e[:, 2 * P:3 * P],
                                                 in0=e[:, 2 * P:3 * P],
                                                 in1=tri_sb)
                            nc.vector.tensor_add(
                                out=esum[:, 2 * P:], in0=esum[:, 2 * P:],
                                in1=e[:, :2 * P])
                            nc.vector.tensor_add(
                                out=esum[:, 3 * P:], in0=esum[:, 3 * P:],
                                in1=e[:, 2 * P:3 * P])
                            nc.tensor.matmul(
                                opd[:, 2 * P:QC],
                                lhsT=v_sb[:, 4 * qc + 2, :],
                                rhs=e[:, :2 * P], start=False, stop=False)
                            nc.tensor.matmul(
                                opd[:, 3 * P:QC],
                                lhsT=v_sb[:, 4 * qc + 3, :],
                                rhs=e[:, 2 * P:3 * P],
                                start=False, stop=True)

                    # cast the f32 esum to bf16 NOW (data just ready, Act
                    # queue short) so the den matmul in the deferred tail
                    # never waits on it; one rounding (~1e-3) on the den
                    esb = espool.tile([P, QC], MMDT, name="esb", tag="esb")
                    nc.scalar.copy(out=esb, in_=esum)

                    def tail():
                        # softmax denominator: broadcast colsum via one
                        # bf16 ones-matmul
                        nc.tensor.matmul(opd[:, QC:], lhsT=ones_sb,
                                         rhs=esb, start=True, stop=True)
                        rb = rbpool.tile([P, QC], F32, name="rb")
                        nc.vector.reciprocal_approx_fast(
                            out=rb, in_=opd[:, QC:])
                        nc.vector.tensor_mul(out=oT[h][:, sl],
                                             in0=opd[:, :QC], in1=rb)
                    return tail

                # Slot schedule: qc0 and qc1 heads interleave at the phase
                # boundary (qc1's bigger units fill qc0's Act-bound gaps --
                # there is no o_proj filler available yet); later chunks
                # run head-sequential with o_proj blocks of the oldest
                # fully-tailed chunk as PE-dense filler between heads.
                slots = []
                for h in range(HQ):
                    slots += [(0, h), (1, h)]
                for qc in range(2, nqc):
                    slots += [(qc, h) for h in range(HQ)]
                ost_fill = {8 + i: i for i in range(8)}  # slot -> o_proj st
                prev_tail = None
                for i, (qc, h) in enumerate(slots):
                    t = emit_units(qc, h, prev_tail)
                    if i in ost_fill:
                        emit_oproj(ost_fill[i], last=False)
                    prev_tail = t
                prev_tail()

                # trailing o_proj for the last two chunks' seq blocks
                for st in range(8, 16):
                    emit_oproj(st, last=(st == 15))

    nc.finalize()
    return nc


def _get_program(mm_mode: str = MM_MODE, s: int = S):
    key = (mm_mode, s)
    if key not in _PROG_CACHE:
        _PROG_CACHE[key] = _build_program(mm_mode, s)
    return _PROG_CACHE[key]


def make_in_maps(hidden_states, cos, sin, Wq, Wk, Wv, Wo, mm_mode=None):
    """Host-side sharding: slice per-core weights, transpose activations."""
    mdt = _mm_np_dtype()
    hidden_states = np.asarray(hidden_states, dtype=np.float32)
    cos = np.asarray(cos, dtype=np.float32)
    sin = np.asarray(sin, dtype=np.float32)
    Wq = np.asarray(Wq, dtype=np.float32)
    Wk = np.asarray(Wk, dtype=np.float32)
    Wv = np.asarray(Wv, dtype=np.float32)
    Wo = np.asarray(Wo, dtype=np.float32)

    s = hidden_states.shape[1]
    nqc, kd, kd4 = s // QC, D // P, D // P // 4
    XT = np.ascontiguousarray(hidden_states[0].T).astype(mdt)  # [D, s]
    # pack X^T so each (q-chunk, 4-contraction-tile) DMA has 4KB contiguous
    # per-partition runs: XP[qc*kd4+a4, p, j*QC+m] = XT[(4*a4+j)*P+p, qc*QC+m]
    XP = np.ascontiguousarray(
        XT.reshape(kd4, 4, P, nqc, QC).transpose(3, 0, 2, 1, 4)
        .reshape(nqc * kd4, P, 4 * QC))
    cT = np.ascontiguousarray(cos[0].T).astype(mdt)            # [HD, s]
    sT = np.ascontiguousarray(sin[0].T).astype(np.float32)
    # sign-folded sin for the DVE rotate-half: rows 0:63 negated
    sTf = np.concatenate([-sT[:HD // 2], sT[HD // 2:]], axis=0).astype(mdt)

    def pack_w(w):
        # [D, m] -> [P, kd*m]: partition p holds rows {a*P+p} concatenated
        m = w.shape[1]
        return np.ascontiguousarray(
            w.reshape(kd, P, m).transpose(1, 0, 2).reshape(P, kd * m))

    kk = np.arange(P)[:, None]
    qq = np.arange(P)[None, :]
    tri = (kk <= qq).astype(np.float32)
    consts = np.concatenate(
        [np.eye(P, dtype=np.float32), np.ones((P, P), np.float32), tri],
        axis=1).astype(mdt)

    in_maps = []
    for c in range(N_CORES):
        cw = c * HQ * HD
        # wo packed like the others but with P-row groups per head:
        # [P, HQ*D]: partition p holds head-h rows {h*P+p}
        wo_c = Wo[cw:cw + HQ * HD, :]
        wo_p = np.ascontiguousarray(
            wo_c.reshape(HQ, P, D).transpose(1, 0, 2).reshape(P, HQ * D))
        wq_p = pack_w(Wq[:, cw:cw + HQ * HD] * np.float32(SCALING)
                      ).astype(mdt)
        wkv_p = pack_w(np.concatenate(
            [Wk[:, c * HD:(c + 1) * HD], Wv[:, c * HD:(c + 1) * HD]],
            axis=1)).astype(mdt)
        m = HQ * HD
        boot = np.ascontiguousarray(np.concatenate(
            [wq_p[:, :m], wkv_p[:, :2 * HD],              # boot0 weights
             wq_p[:, m:4 * m], wkv_p[:, 2 * HD:8 * HD],   # boot1a
             consts, wkv_p[:, 8 * HD:32 * HD]],           # boot1b
            axis=1))
        assert boot.shape[1] == BOOT_COLS, boot.shape
        in_maps.append({
            "xt": XP,
            "wq": wq_p,
            "wkv": wkv_p,
            "wo": wo_p.astype(mdt),
            "cost": cT,
            "sint": sTf,
            "boot": boot,
        })
    return in_maps


def run_spmd(in_maps, s: int = S, trace: bool = False, **kw):
    from concourse.bass_utils import run_bass_kernel_spmd

    nc = _get_program(MM_MODE, s)
    return run_bass_kernel_spmd(
        nc, in_maps, core_ids=list(range(N_CORES)), trace=trace, **kw
    )


def kernel(hidden_states, cos, sin, Wq, Wk, Wv, Wo):
    in_maps = make_in_maps(hidden_states, cos, sin, Wq, Wk, Wv, Wo)
    s = np.asarray(hidden_states).shape[1]
    res = run_spmd(in_maps, s=s, trace=False)
    total = np.zeros((s, D), np.float64)
    for r in res.results:
        total += np.asarray(r["out"], dtype=np.float32)
    return total.astype(np.float32).reshape(1, s, D)


# revision 31
# speedup vs baseline: 1.0091x; 1.0083x over previous
"""Llama GQA attention (B=1, S=2048, D=4096, H=32, KV=8, HD=128) on 8 Trainium2
NeuronCores, tensor-parallel over heads.

Sharding: core c owns Q heads 4c..4c+3 and KV head c (GQA groups align with the
8 KV heads). Wq/Wk/Wv are column-sliced, Wo row-sliced; each core produces a
full-shape partial output (bf16) and the host sums the 8 partials (row-parallel
TP all-reduce done at unshard time).

Device kernel layout strategy: the host passes X^T so every projection matmul
produces transposed activations [head_dim=128 partitions, seq free]:
    Q^T/K^T/V^T = W.T @ X^T   (lhsT = W slice, rhs = X^T tile)
Scores are computed transposed, S^T[k, q] = K^T_tile.T @ Q^T, the exp runs on
the scalar engine (PSUM->SBUF), and the PV matmul consumes E^T directly
(lhsT = V natural tile); o_proj consumes O^T directly as lhsT. RoPE
rotate-half is done with two small SBUF->SBUF DMAs (DVE lanes are
partition-locked; DMA is not) plus one DVE multiply against a sign-folded
sin table (no PE matmul). 1/sqrt(HD) is folded into Wq on the host.
Causality: k-tiles above the diagonal are skipped, and on the four
diagonal k-tiles the q range is trimmed to q >= j*128 (N = 512,384,256,128),
with one shared [128,128] triangular multiplicative mask on each tile's
leading 128 q columns (exp never overflows: scores are O(10) here, so
max-subtraction is unnecessary). Diagonal scores share PSUM banks so a
single exp covers each two-tile group.

Softmax denominator: instead of a PE ones-matmul per k-tile (which costs as
much as PV again), the DVE accumulates esum[q] = sum over processed e-tiles
in fp32 SBUF (2 adds per pair), the scalar engine casts it to bf16 right
when it completes, and ONE bf16 ones@esum matmul per (q-chunk, head)
broadcasts the denominator across partitions. This cuts attention PE time
by ~30%. Each head's den/recip/mul tail is deferred one slot so the den
matmul never stalls the in-order PE stream on the DVE chain.

Slot schedule: qc0 and qc1 heads interleave at the phase boundary (qc1's
bigger units fill qc0's Act-bound pipeline-fill gaps), later chunks run
head-sequential with o_proj blocks of the oldest finished chunk as PE-dense
filler between heads (attention alone is Act-exp-bound once den is off the
PE), and the last two chunks' o_proj trails as a pure-PE tail.

Scheduling (what got this from 526us to ~426us and then ~409us):
 - All DRAM tensors are host-packed so every DMA has 4-8KB contiguous
   per-partition runs (descriptor size = per-partition run length).
 - 18 warm-up matmuls on a memset tile run from t~6us so the PE HAM clock
   gate (4/8 -> 8/8 at ~3.4us of sustained activity) is already open when
   the first real matmul's operands land.
 - boot DMA is split: boot0 (consts + a=0 weights, ~330KB) unblocks the
   first matmul; boot1a/boot1b ride behind, interleaved with the X quarters.
   The wq/wkv streams are chopped into ~0.5MB pieces spread across the
   first x-tile iterations so the startup dependency set isn't diluted by
   bulk prefetch (xin pool depth 6 bounds in-flight X).
 - Attention processes full k-tile PAIRS: scores land in a [128,1024] PSUM
   tile and one scalar-engine exp covers both tiles, amortizing the ACTIVATE
   fixed cost (352cyc).
 - Attention is qc-outer/head-inner and o_proj for the finished 512-row block
   is emitted right after, so phases 2+3 form one dense PE stream; PSUM is
   exactly 8 banks: 2x score-pair + 2x (ops|den).
 - epool/rbpool/espool live on the RIGHT side of SBUF so the first exp's
   e-tile never WARs against phase-1 regions still being read by the last
   RoPE epilogue (this WAR caused a 2.5us PE bubble + 10us HAM re-throttle
   at the phase boundary).
 - Each RoPE epilogue runs its PSUM readers (casts, V transposes) FIRST and
   defers the SBUF-only muls/adds, so acc banks free early for attention.
 - softmax reciprocal uses the single-op DVE reciprocal_approx_fast.
 - o_proj results stage through [128,2048] tiles (512KB DMAs, 4KB runs) and
   partials are written as bf16 (host sums in f64); the very last staging
   tile is split into two 256KB DMAs so the NEFF drain starts earlier.

Matmul operands are bf16 (PE runs 4x faster than true fp32; accumulation stays
fp32 in PSUM); softmax statistics and RoPE trig stay fp32. fp8 was evaluated
and rejected: e4m3 quantization noise on any of the big projections costs
3-6e-2 rel err vs the 2e-2 budget.
"""

import numpy as np

S = 2048
D = 4096
HD = 128
HQ = 4            # Q heads per core
P = 128
QC = 512          # q-chunk (matmul moving free dim)
SCALING = float(HD) ** -0.5
N_CORES = 8
WARM_MM = 18      # PE warm-up matmuls (HAM un-throttle before real work)

MM_MODE = "bf16"

_PROG_CACHE = {}


def _mm_np_dtype(mm_mode="bf16"):
    import ml_dtypes
    return ml_dtypes.bfloat16


# boot column layout (bf16): wq a=0 | wkv a=0 | wq a=1..3 | wkv a=1..3 |
#   id | ones | tri | wkv a=4..15   (weights first: the consts aren't
#   needed until the first epilogue, so only 192KB gates the first matmul)
BOOT0_END = HQ * HD + 2 * HD                  # 768
BOOT1A_END = BOOT0_END + 3 * HQ * HD + 3 * 2 * HD   # 3072
CONST0 = BOOT1A_END                           # id|ones|tri
BOOT_COLS = CONST0 + 3 * P + 12 * 2 * HD      # 6528


def _build_program(mm_mode: str = "bf16", s: int = S):
    import concourse.tile as tile
    from concourse import bacc, mybir

    F32 = mybir.dt.float32
    F32R = mybir.dt.float32r
    BF16 = mybir.dt.bfloat16
    MMDT = BF16
    H2 = HD // 2

    nqc = s // QC           # q chunks
    nkt = s // P            # k tiles
    kd = D // P             # contraction tiles over model dim

    kd4 = kd // 4           # packed X groups of 4 contraction tiles

    nc = bacc.Bacc("TRN2", target_bir_lowering=False, debug=False)
    xt = nc.dram_tensor("xt", [nqc * kd4, P, 4 * QC], MMDT,
                        kind="ExternalInput")
    wq = nc.dram_tensor("wq", [P, kd * HQ * HD], MMDT, kind="ExternalInput")
    wkv = nc.dram_tensor("wkv", [P, kd * 2 * HD], MMDT, kind="ExternalInput")
    wo = nc.dram_tensor("wo", [P, HQ * D], MMDT, kind="ExternalInput")
    cost = nc.dram_tensor("cost", [HD, s], MMDT, kind="ExternalInput")
    sint = nc.dram_tensor("sint", [HD, s], MMDT, kind="ExternalInput")
    boot = nc.dram_tensor("boot", [P, BOOT_COLS], MMDT, kind="ExternalInput")
    out = nc.dram_tensor("out", [s, D], MMDT, kind="ExternalOutput")

    wq_r = wq.ap().rearrange("p (a m) -> p a m", m=HQ * HD)  # [128, kd, 512]
    wkv_r = wkv.ap().rearrange("p (a m) -> p a m", m=2 * HD)
    wo_r = wo.ap().rearrange("p (h d) -> p h d", d=D)        # [128, HQ, D]
    out_r = out.ap().rearrange("(a p) d -> a p d", p=P)      # [s/128, 128, D]

    def boot_wq_off(a):
        return 0 if a == 0 else BOOT0_END + (a - 1) * HQ * HD

    def boot_wkv_off(a):
        if a == 0:
            return HQ * HD
        if a < 4:
            return BOOT0_END + 3 * HQ * HD + (a - 1) * 2 * HD
        return CONST0 + 3 * P + (a - 4) * 2 * HD

    with tile.TileContext(nc) as tc:
        with tc.tile_pool(name="persist", bufs=1) as persist:
            qT = [persist.tile([HD, s], MMDT, name=f"qT{h}") for h in range(HQ)]
            kT = persist.tile([HD, s], MMDT, name="kT")
            v_sb = persist.tile([P, nkt, HD], MMDT, name="v_sb")
            oT = [persist.tile([HD, s], MMDT, name=f"oT{h}") for h in range(HQ)]
            wo_sb = persist.tile([P, HQ, D], MMDT, name="wo_sb")
            ones32 = persist.tile([P, P], F32, name="ones32")
            boot_sb = persist.tile([P, BOOT_COLS], MMDT, name="boot_sb")
            id_sb = boot_sb[:, CONST0:CONST0 + P]
            ones_sb = boot_sb[:, CONST0 + P:CONST0 + 2 * P]
            tri_sb = boot_sb[:, CONST0 + 2 * P:CONST0 + 3 * P]

            # ---------------- Phase 1: QKV projection + RoPE ----------------
            with (
                tc.tile_pool(name="ph1", bufs=1) as ph1,
                tc.tile_pool(name="xin", bufs=6) as xin,
                tc.tile_pool(name="ropes", bufs=3) as ropes,
                tc.tile_pool(name="accp", bufs=1, space="PSUM") as accp,
                tc.tile_pool(name="rqp", bufs=2, space="PSUM") as rqp,
            ):
                cos_sb = ph1.tile([HD, s], MMDT, name="cos_sb")
                sin_sb = ph1.tile([HD, s], MMDT, name="sin_sb")
                vT_sb = ph1.tile([HD, s], MMDT, name="vT_sb")
                wq_sb = ph1.tile([P, kd, HQ * HD], MMDT, name="wq_sb")
                wkv_sb = ph1.tile([P, kd, 2 * HD], MMDT, name="wkv_sb")

                # PE warm-up: HAM opens the clock gate after ~3.4us of
                # sustained activity; run it on a memset tile while the
                # boot DMA is in flight so real matmuls start at 2.4GHz.
                warm_sb = ph1.tile([P, QC], MMDT, name="warm_sb")
                nc.vector.memset(warm_sb, 0.0)
                nc.vector.memset(ones32, 1.0)
                for _ in range(WARM_MM):
                    wp = rqp.tile([P, QC], F32, name="wp", tag="rq")
                    nc.tensor.matmul(wp, lhsT=warm_sb[:, :P], rhs=warm_sb,
                                     start=True, stop=True)

                for ci, qc in enumerate(range(nqc)):
                    sl = slice(qc * QC, (qc + 1) * QC)
                    accs = [
                        accp.tile([P, QC], F32, name=f"acc{t}", tag=f"acc{t}")
                        for t in range(6)
                    ]
                    for a4 in range(kd4):
                        xt_t = xin.tile([P, 4 * QC], MMDT, name="xt_t")
                        if ci == 0 and a4 == 0:
                            # startup: interleave the first-matmul dependency
                            # set (boot0 + first x quarter) ahead of the rest
                            nc.sync.dma_start(boot_sb[:, :BOOT0_END],
                                              boot.ap()[:, :BOOT0_END])
                            nc.sync.dma_start(xt_t[:, :QC],
                                              xt.ap()[0][:, :QC])
                            nc.sync.dma_start(
                                boot_sb[:, BOOT0_END:BOOT1A_END],
                                boot.ap()[:, BOOT0_END:BOOT1A_END])
                            nc.sync.dma_start(xt_t[:, QC:2 * QC],
                                              xt.ap()[0][:, QC:2 * QC])
                            nc.sync.dma_start(boot_sb[:, BOOT1A_END:],
                                              boot.ap()[:, BOOT1A_END:])
                            nc.sync.dma_start(xt_t[:, 2 * QC:],
                                              xt.ap()[0][:, 2 * QC:])
                        else:
                            nc.sync.dma_start(xt_t, xt.ap()[qc * kd4 + a4])
                        if ci == 0 and a4 <= 6:
                            # rest of wq in 0.5MB pieces between the x tiles
                            c = 4 + 4 * a4
                            nc.sync.dma_start(wq_sb[:, c:c + 4, :],
                                              wq_r[:, c:c + 4, :])
                        if ci == 0 and a4 in (2, 3):
                            c = 16 + 8 * (a4 - 2)
                            nc.sync.dma_start(wkv_sb[:, c:c + 8, :],
                                              wkv_r[:, c:c + 8, :])
                        if ci == 0 and a4 == 5:
                            nc.sync.dma_start(cos_sb, cost.ap())
                            nc.sync.dma_start(sin_sb, sint.ap())
                        if ci == 1 and a4 in (1, 3, 5, 7):
                            # phase-2/3 constants, spread out mid-stream
                            h = (a4 - 1) // 2
                            nc.sync.dma_start(wo_sb[:, h, :], wo_r[:, h, :])
                        for j in range(4):
                            a = 4 * a4 + j
                            rhs = xt_t[:, j * QC:(j + 1) * QC]
                            if a < 4:
                                qo = boot_wq_off(a)
                                qa = boot_sb[:, qo:qo + HQ * HD]
                            else:
                                qa = wq_sb[:, a, :]
                            wsl = [qa[:, h * HD:(h + 1) * HD]
                                   for h in range(HQ)]
                            if a < 16:
                                kb = boot_wkv_off(a)
                                wsl += [boot_sb[:, kb:kb + HD],
                                        boot_sb[:, kb + HD:kb + 2 * HD]]
                            else:
                                wsl += [wkv_sb[:, a, 0:HD],
                                        wkv_sb[:, a, HD:]]
                            for t in range(6):
                                nc.tensor.matmul(
                                    accs[t], lhsT=wsl[t], rhs=rhs,
                                    start=(a == 0), stop=(a == kd - 1),
                                )
                    # RoPE epilogue: PSUM readers (casts, V transposes) run
                    # FIRST so the acc banks (aliased by the attention PSUM
                    # pool) free early; the SBUF-only muls/adds are deferred
                    # and overlap with whatever follows.
                    nc.scalar.copy(out=vT_sb[:, sl], in_=accs[5])
                    raws = []
                    for t in range(5):
                        raw = ropes.tile([P, QC], MMDT, name="raw",
                                         tag="raw", bufs=5)
                        if t % 2 == 1 or t == 4:
                            nc.scalar.copy(out=raw, in_=accs[t])
                        else:
                            nc.vector.tensor_copy(out=raw, in_=accs[t])
                        raws.append(raw)
                    # V^T -> V natural layout for this chunk's 4 seq tiles
                    for st in range(4 * qc, 4 * qc + 4):
                        tp = rqp.tile([P, P], MMDT, name="tp", tag="rq")
                        nc.tensor.transpose(tp, vT_sb[:, st * P:(st + 1) * P],
                                            id_sb)
                        nc.vector.tensor_copy(out=v_sb[:, st, :], in_=tp)
                    if ci == nqc - 1:
                        # pad the PE through the last epilogue's lull so the
                        # HAM clock gate stays at 8/8 into the attention
                        # phase (it re-throttles after ~3.4us of idle)
                        for _ in range(14):
                            wpad = rqp.tile([P, QC], F32, name="wpad",
                                            tag="rq")
                            nc.tensor.matmul(wpad, lhsT=ones_sb,
                                             rhs=boot_sb[:, :QC],
                                             start=True, stop=True)
                    # SBUF-only tail: rotate-half via two small SBUF->SBUF
                    # DMAs (DVE lanes are partition-locked, DMA is not),
                    # then one mul against the sign-folded sin (rows 0:64
                    # negated host-side) -- no PE matmul needed.
                    for t in range(5):
                        rot = ropes.tile([P, QC], MMDT, name="rot",
                                         tag="rot", bufs=5)
                        nc.sync.dma_start(rot[0:H2], raws[t][H2:P])
                        nc.sync.dma_start(rot[H2:P], raws[t][0:H2])
                        tmp = ropes.tile([P, QC], F32, name="tmp",
                                         tag="tmp", bufs=5)
                        nc.vector.tensor_mul(out=tmp, in0=rot,
                                             in1=sin_sb[:, sl])
                        dst = qT[t] if t < HQ else kT
                        nc.vector.tensor_mul(out=dst[:, sl], in0=raws[t],
                                             in1=cos_sb[:, sl])
                        nc.vector.tensor_add(out=dst[:, sl], in0=dst[:, sl],
                                             in1=tmp)

            # ---- Phases 2+3 software-pipelined: attention + o_proj ----
            # PSUM: sp tag = 2x [128,1024] (score units AND o_proj dd-pairs,
            #       4 banks), od tag = 2x [128,1024] opd ops|den (4 banks).
            # epool/rbpool/espool are on the RIGHT side of SBUF: no WAR
            # against phase-1 regions still read by the last RoPE tail.
            # Each head's den/recip/mul tail is DEFERRED until after the
            # next head's units (the den matmul otherwise stalls the
            # in-order PE stream on the DVE esum chain), and o_proj blocks
            # of chunk qc-1 are interleaved between chunk qc's heads so the
            # PE has dense filler while the scalar engine works through the
            # exps (attention alone is Act-bound once den is off the PE).
            with (
                tc.tile_pool(name="ppsum", bufs=2, space="PSUM") as ppsum,
                tc.tile_pool(name="epool", bufs=3, side="right") as epool,
                tc.tile_pool(name="rbpool", bufs=2, side="right") as rbpool,
                tc.tile_pool(name="espool", bufs=2, side="right") as espool,
                tc.tile_pool(name="res", bufs=4) as res,
            ):
                def emit_oproj(st, last):
                    # o_proj for one finished 128-row seq block (both halves)
                    for half in range(2):
                        r = res.tile([P, 4 * QC], MMDT, name="r")
                        for k in range(2):
                            ddp = 2 * half + k
                            op = ppsum.tile([P, 2 * QC], F32, name="op",
                                            tag="sp")
                            for i in range(2):
                                c0 = ddp * 2 * QC + i * QC
                                for h in range(HQ):
                                    nc.tensor.matmul(
                                        op[:, i * QC:(i + 1) * QC],
                                        lhsT=oT[h][:, st * P:(st + 1) * P],
                                        rhs=wo_sb[:, h, c0:c0 + QC],
                                        start=(h == 0), stop=(h == HQ - 1),
                                    )
                            dst = r[:, k * 2 * QC:(k + 1) * 2 * QC]
                            if last and half == 1:
                                # final tiles: split copies across both
                                # engines and DMA each 256KB piece as soon
                                # as it's staged (shortens the drain)
                                nc.vector.tensor_copy(out=dst[:, :QC],
                                                      in_=op[:, :QC])
                                nc.scalar.copy(out=dst[:, QC:],
                                               in_=op[:, QC:])
                                nc.sync.dma_start(
                                    out_r[st, :,
                                          (half * 2 + k) * 2 * QC:
                                          (half * 2 + k + 1) * 2 * QC],
                                    dst)
                            elif k == 0:
                                nc.vector.tensor_copy(out=dst, in_=op)
                            else:
                                # k1 on the scalar engine: balancing the
                                # PSUM-read copies across engines keeps the
                                # DVE FIFO short for the recip/mul tails
                                nc.scalar.copy(out=dst, in_=op)
                        if not (last and half == 1):
                            nc.sync.dma_start(
                                out_r[st, :,
                                      half * 4 * QC:(half + 1) * 4 * QC],
                                r)

                def emit_units(qc, h):
                    """Score units + exps + masks + esum + PVs for one
                    (q-chunk, head); returns the deferred tail closure."""
                    sl = slice(qc * QC, (qc + 1) * QC)
                    nfull = 2 * qc          # full (unmasked) k-tile pairs
                    nunit = nfull + 2       # + 2 trimmed diagonal groups

                    def qk_pair(g):
                        sp = ppsum.tile([P, 2 * QC], F32, name="sp",
                                        tag="sp")
                        for i in range(2):
                            kt = 2 * g + i
                            nc.tensor.matmul(
                                sp[:, i * QC:(i + 1) * QC],
                                lhsT=kT[:, kt * P:(kt + 1) * P],
                                rhs=qT[h][:, sl], start=True, stop=True,
                            )
                        return sp

                    def qk_diag(which):
                        # diagonal k-tiles with q trimmed to q >= j*128:
                        # which=0: j=0 (N=512 at cols 0:512),
                        #          j=1 (N=384 at cols 512:896)
                        # which=1: j=2 (N=256), j=3 (N=128) -- written into
                        #          opd's den bank (dead until the tail's den
                        #          matmul overwrites it), so the diag-2
                        #          group consumes NO sp-ring slot and every
                        #          later sp WAR shifts one exp earlier.
                        #          Two start=True writes to one bank only
                        #          clear has_written; data is preserved, so
                        #          one contiguous exp covers both.
                        if which == 0:
                            sp = ppsum.tile([P, 2 * QC], F32, name="sp",
                                            tag="sp")
                        else:
                            sp = opd[:, QC:]
                        for j in (0, 1) if which == 0 else (2, 3):
                            n = QC - j * P
                            kt = 4 * qc + j
                            off = {0: 0, 1: QC, 2: 0, 3: 2 * P}[j]
                            nc.tensor.matmul(
                                sp[:, off:off + n],
                                lhsT=kT[:, kt * P:(kt + 1) * P],
                                rhs=qT[h][:, qc * QC + j * P:(qc + 1) * QC],
                                start=True, stop=True,
                                skip_group_check=(which == 1),
                            )
                        return sp

                    def unit_scores(u):
                        if u < nfull:
                            return qk_pair(u)
                        return qk_diag(u - nfull)

                    opd = ppsum.tile([P, 2 * QC], F32, name="opd",
                                     tag="od")
                    esum = espool.tile([P, QC], F32, name="esum")
                    sps = [unit_scores(0), unit_scores(1)]
                    for u in range(nunit):
                        if u + 2 < nunit:
                            sps.append(unit_scores(u + 2))
                        sp = sps[u]
                        e = epool.tile([P, 2 * QC], MMDT, name="e")
                        if u < nfull:
                            nc.scalar.activation(
                                out=e, in_=sp,
                                func=mybir.ActivationFunctionType.Exp,
                            )
                            if u == 0:
                                nc.vector.tensor_add(out=esum,
                                                     in0=e[:, :QC],
                                                     in1=e[:, QC:])
                            else:
                                nc.vector.tensor_add(out=esum, in0=esum,
                                                     in1=e[:, :QC])
                                nc.vector.tensor_add(out=esum, in0=esum,
                                                     in1=e[:, QC:])
                            for i in range(2):
                                kt = 2 * u + i
                                nc.tensor.matmul(
                                    opd[:, :QC], lhsT=v_sb[:, kt, :],
                                    rhs=e[:, i * QC:(i + 1) * QC],
                                    start=(u == 0 and i == 0), stop=False,
                                )
                        elif u == nfull:
                            # diag group 1: j=0 (N=512), j=1 (N=384);
                            # one exp covers both (cols 0:896 contiguous)
                            nc.scalar.activation(
                                out=e[:, :QC + 3 * P],
                                in_=sp[:, :QC + 3 * P],
                                func=mybir.ActivationFunctionType.Exp)
                            nc.vector.tensor_mul(out=e[:, :P],
                                                 in0=e[:, :P], in1=tri_sb)
                            nc.vector.tensor_mul(out=e[:, QC:QC + P],
                                                 in0=e[:, QC:QC + P],
                                                 in1=tri_sb)
                            if u == 0:
                                nc.vector.tensor_copy(out=esum,
                                                      in_=e[:, :QC])
                            else:
                                nc.vector.tensor_add(out=esum, in0=esum,
                                                     in1=e[:, :QC])
                            nc.vector.tensor_add(
                                out=esum[:, P:], in0=esum[:, P:],
                                in1=e[:, QC:QC + 3 * P])
                            nc.tensor.matmul(
                                opd[:, :QC], lhsT=v_sb[:, 4 * qc, :],
                                rhs=e[:, :QC],
                                start=(nfull == 0), stop=False)
                            nc.tensor.matmul(
                                opd[:, P:QC], lhsT=v_sb[:, 4 * qc + 1, :],
                                rhs=e[:, QC:QC + 3 * P],
                                start=False, stop=False)
                        else:
                            # diag group 2: j=2 (N=256 at 0:256), j=3
                            # (N=128 at 256:384); one exp covers both
                            nc.scalar.activation(
                                out=e[:, :3 * P], in_=sp[:, :3 * P],
                                func=mybir.ActivationFunctionType.Exp)
                            nc.vector.tensor_mul(out=e[:, :P],
                                                 in0=e[:, :P], in1=tri_sb)
                            nc.vector.tensor_mul(out=e[:, 2 * P:3 * P],
                                                 in0=e[:, 2 * P:3 * P],
                                                 in1=tri_sb)
                            nc.vector.tensor_add(
                                out=esum[:, 2 * P:], in0=esum[:, 2 * P:],
                                in1=e[:, :2 * P])
                            nc.vector.tensor_add(
                                out=esum[:, 3 * P:], in0=esum[:, 3 * P:],
                                in1=e[:, 2 * P:3 * P])
                            nc.tensor.matmul(
                                opd[:, 2 * P:QC],
                                lhsT=v_sb[:, 4 * qc + 2, :],
                                rhs=e[:, :2 * P], start=False, stop=False)
                            nc.tensor.matmul(
                                opd[:, 3 * P:QC],
                                lhsT=v_sb[:, 4 * qc + 3, :],
                                rhs=e[:, 2 * P:3 * P],
                                start=False, stop=True)

                    # cast the f32 esum to bf16 NOW (data just ready, Act
                    # queue short) so the den matmul in the deferred tail
                    # never waits on it; one rounding (~1e-3) on the den
                    esb = espool.tile([P, QC], MMDT, name="esb", tag="esb")
                    nc.scalar.copy(out=esb, in_=esum)

                    def tail():
                        # softmax denominator: broadcast colsum via one
                        # bf16 ones-matmul (overwrites the diag-2 scratch
                        # in opd's den bank; exp has long since read it)
                        nc.tensor.matmul(opd[:, QC:], lhsT=ones_sb,
                                         rhs=esb, start=True, stop=True,
                                         skip_group_check=True)
                        rb = rbpool.tile([P, QC], F32, name="rb")
                        nc.vector.reciprocal_approx_fast(
                            out=rb, in_=opd[:, QC:])
                        nc.vector.tensor_mul(out=oT[h][:, sl],
                                             in0=opd[:, :QC], in1=rb)
                    return tail

                # Slot schedule: qc0 and qc1 heads interleave at the phase
                # boundary (qc1's bigger units fill qc0's Act-bound gaps --
                # there is no o_proj filler available yet); later chunks
                # run head-sequential with o_proj blocks of the oldest
                # fully-tailed chunk as PE-dense filler between heads.
                slots = []
                for h in range(HQ):
                    slots += [(0, h), (1, h)]
                for qc in range(2, nqc):
                    slots += [(qc, h) for h in range(HQ)]
                ost_fill = {8 + i: i for i in range(8)}  # slot -> o_proj st
                prev_tail = None
                for i, (qc, h) in enumerate(slots):
                    t = emit_units(qc, h)
                    if prev_tail is not None:
                        prev_tail()
                    if i in ost_fill:
                        emit_oproj(ost_fill[i], last=False)
                    prev_tail = t
                prev_tail()

                # trailing o_proj for the last two chunks' seq blocks
                for st in range(8, 16):
                    emit_oproj(st, last=(st == 15))

    nc.finalize()
    return nc


def _get_program(mm_mode: str = MM_MODE, s: int = S):
    key = (mm_mode, s)
    if key not in _PROG_CACHE:
        _PROG_CACHE[key] = _build_program(mm_mode, s)
    return _PROG_CACHE[key]


def make_in_maps(hidden_states, cos, sin, Wq, Wk, Wv, Wo, mm_mode=None):
    """Host-side sharding: slice per-core weights, transpose activations."""
    mdt = _mm_np_dtype()
    hidden_states = np.asarray(hidden_states, dtype=np.float32)
    cos = np.asarray(cos, dtype=np.float32)
    sin = np.asarray(sin, dtype=np.float32)
    Wq = np.asarray(Wq, dtype=np.float32)
    Wk = np.asarray(Wk, dtype=np.float32)
    Wv = np.asarray(Wv, dtype=np.float32)
    Wo = np.asarray(Wo, dtype=np.float32)

    s = hidden_states.shape[1]
    nqc, kd, kd4 = s // QC, D // P, D // P // 4
    XT = np.ascontiguousarray(hidden_states[0].T).astype(mdt)  # [D, s]
    # pack X^T so each (q-chunk, 4-contraction-tile) DMA has 4KB contiguous
    # per-partition runs: XP[qc*kd4+a4, p, j*QC+m] = XT[(4*a4+j)*P+p, qc*QC+m]
    XP = np.ascontiguousarray(
        XT.reshape(kd4, 4, P, nqc, QC).transpose(3, 0, 2, 1, 4)
        .reshape(nqc * kd4, P, 4 * QC))
    cT = np.ascontiguousarray(cos[0].T).astype(mdt)            # [HD, s]
    sT = np.ascontiguousarray(sin[0].T).astype(np.float32)
    # sign-folded sin for the DVE rotate-half: rows 0:63 negated
    sTf = np.concatenate([-sT[:HD // 2], sT[HD // 2:]], axis=0).astype(mdt)

    def pack_w(w):
        # [D, m] -> [P, kd*m]: partition p holds rows {a*P+p} concatenated
        m = w.shape[1]
        return np.ascontiguousarray(
            w.reshape(kd, P, m).transpose(1, 0, 2).reshape(P, kd * m))

    kk = np.arange(P)[:, None]
    qq = np.arange(P)[None, :]
    tri = (kk <= qq).astype(np.float32)
    consts = np.concatenate(
        [np.eye(P, dtype=np.float32), np.ones((P, P), np.float32), tri],
        axis=1).astype(mdt)

    in_maps = []
    for c in range(N_CORES):
        cw = c * HQ * HD
        # wo packed like the others but with P-row groups per head:
        # [P, HQ*D]: partition p holds head-h rows {h*P+p}
        wo_c = Wo[cw:cw + HQ * HD, :]
        wo_p = np.ascontiguousarray(
            wo_c.reshape(HQ, P, D).transpose(1, 0, 2).reshape(P, HQ * D))
        wq_p = pack_w(Wq[:, cw:cw + HQ * HD] * np.float32(SCALING)
                      ).astype(mdt)
        wkv_p = pack_w(np.concatenate(
            [Wk[:, c * HD:(c + 1) * HD], Wv[:, c * HD:(c + 1) * HD]],
            axis=1)).astype(mdt)
        m = HQ * HD
        boot = np.ascontiguousarray(np.concatenate(
            [wq_p[:, :m], wkv_p[:, :2 * HD],              # boot0 weights
             wq_p[:, m:4 * m], wkv_p[:, 2 * HD:8 * HD],   # boot1a
             consts, wkv_p[:, 8 * HD:32 * HD]],           # boot1b
            axis=1))
        assert boot.shape[1] == BOOT_COLS, boot.shape
        in_maps.append({
            "xt": XP,
            "wq": wq_p,
            "wkv": wkv_p,
            "wo": wo_p.astype(mdt),
            "cost": cT,
            "sint": sTf,
            "boot": boot,
        })
    return in_maps


def run_spmd(in_maps, s: int = S, trace: bool = False, **kw):
    from concourse.bass_utils import run_bass_kernel_spmd

    nc = _get_program(MM_MODE, s)
    return run_bass_kernel_spmd(
        nc, in_maps, core_ids=list(range(N_CORES)), trace=trace, **kw
    )


def kernel(hidden_states, cos, sin, Wq, Wk, Wv, Wo):
    in_maps = make_in_maps(hidden_states, cos, sin, Wq, Wk, Wv, Wo)
    s = np.asarray(hidden_states).shape[1]
    res = run_spmd(in_maps, s=s, trace=False)
    total = np.zeros((s, D), np.float64)
    for r in res.results:
        total += np.asarray(r["out"], dtype=np.float32)
    return total.astype(np.float32).reshape(1, s, D)
